# revision 112
# baseline (speedup 1.0000x reference)
"""Trainium2 Bass kernel: transformer block (LN->attn->LN->MLP, pre-norm residual).

Sharding: 8 cores, zero collectives. Core c handles batch b=c//2, query-token
half h=c%2 (1024 q-tokens). Each core computes LN1 + K/V over its batch's full
2048 tokens (duplicated within the pair), Q/attention/proj/MLP only for its
1024 tokens. Host rolls tokens so the q-half is always tokens 0..1023 (softmax
is permutation-invariant over keys), keeping one SPMD program for all cores.

v2 numerics: the attention half runs in fp8e4m3 with DoubleRow matmuls
(two contraction rows per partition, 0.5 cycles/row on PE):
  - LN1 output quantized to fp8; QKV/score/AV/proj matmuls all DoubleRow.
  - exp(S) fits e4m3's normal range directly (|S| <= ~2.8), so softmax
    numerators are stored fp8 unshifted; denominators accumulate in fp32
    PSUM via a ones column in the V tile.
  - per head the 64 score-contraction dims live on 32 partitions x 2
    DoubleRow slots; 4 heads stack on 128 partitions.
  - AV wants keys as [64 partitions x 2 slots]; exp output is folded into
    that layout by SBUF->SBUF DMAs (the DMA engines are otherwise idle).
The MLP stays bf16 (fp8 there fails the 2e-2 gate). Biases/LN affine fold
on the host as in v1.

Scheduling: MLP(qc0) is emitted between qc1's attention groups so PE stays
busy while ACT streams exp; activation-table swaps are minimized by keeping
phase A on the sqrt set (LN1 + Identity evicts), groups on the exp set
(LN2's rsqrt is exp(-0.5*ln(v+eps))), and batching gelus.
"""

import numpy as np
import ml_dtypes
from contextlib import ExitStack

import concourse.bass as bass
import concourse.tile as tile
from concourse import bacc, mybir
from concourse.bass_utils import run_bass_kernel_spmd

F32 = mybir.dt.float32
BF16 = mybir.dt.bfloat16
F8 = mybir.dt.float8e4
AF = mybir.ActivationFunctionType
ALU = mybir.AluOpType
DR = mybir.MatmulPerfMode.DoubleRow
E4 = ml_dtypes.float8_e4m3

DIM = 768
NH = 12
HD = 64
HID = 3072
B = 4
T = 2048
TQ = 1024
NCORES = 8
EPS = 1e-6

KC = DIM // 128     # 6  contraction chunks over model dim
KP = KC // 2        # 3  DoubleRow pairs over model dim
HC = HID // 128     # 24 contraction chunks over hidden dim
NTB = T // 128      # 16 token blocks (full batch)
NQB = TQ // 128     # 8  q-token blocks
HP = NH // 2        # 6  head pairs

DEBUG_DUMPS = False


def _emit(nc, tc, ctx, d):
    P = 128

    def dump(name, ap):
        if DEBUG_DUMPS:
            nc.sync.dma_start(d[name], ap)

    # ---- whole-kernel pools ----
    outp = ctx.enter_context(tc.tile_pool(name="outer", bufs=1))
    statp = ctx.enter_context(tc.tile_pool(name="stats", bufs=8))
    yop = ctx.enter_context(tc.tile_pool(name="yout", bufs=1))

    ones1 = outp.tile([1, P], BF16, tag="ones1")
    ident16 = outp.tile([P, P], BF16, tag="ident16")
    epst = outp.tile([P, 1], F32, tag="epst")
    bfc2 = outp.tile([1, DIM], BF16, tag="bfc2")
    y1 = outp.tile([P, NQB, DIM], BF16, tag="y1")  # bf16 residual stream

    nc.gpsimd.memset(ones1[:, :], 1.0)
    nc.gpsimd.memset(epst[:, :], EPS)
    sqwarm = outp.tile([1, 1], F32, tag="sqwarm")
    nc.scalar.activation(sqwarm[:, :], epst[0:1, :], AF.Sqrt)
    nc.sync.dma_start(bfc2[:, :], d["bfc2"])
    nc.sync.dma_start(ident16[:, :], d["ident16"])

    def ln_stats(src_ap, use_ln_exp):
        """mean/var -> (rs, negmurs) [P,1] scalars for the affine evict."""
        st = statp.tile([P, 2, 6], F32, tag="st")
        nc.vector.bn_stats(st[:, 0, :], src_ap[:, 0:384])
        nc.vector.bn_stats(st[:, 1, :], src_ap[:, 384:768])
        ag = statp.tile([P, 2], F32, tag="ag")
        nc.vector.bn_aggr(ag[:], st[:])
        rs = statp.tile([P, 1], F32, tag="rs")
        if use_ln_exp:
            # rsqrt via the exp table set: rs = exp(-0.5*ln(var+eps))
            lnv = statp.tile([P, 1], F32, tag="lnv")
            nc.scalar.activation(lnv[:], ag[:, 1:2], AF.Ln, bias=epst[:, :])
            nc.scalar.activation(rs[:], lnv[:], AF.Exp, scale=-0.5)
        else:
            sd = statp.tile([P, 1], F32, tag="sd")
            nc.scalar.activation(sd[:], ag[:, 1:2], AF.Sqrt, bias=epst[:, :])
            nc.vector.reciprocal(rs[:], sd[:])
        nm = statp.tile([P, 1], F32, tag="nm")
        nc.vector.scalar_tensor_tensor(nm[:], ag[:, 0:1], -1.0, rs[:],
                                       ALU.mult, ALU.mult)
        return rs, nm

    # ================= attention half =================
    with tc.tile_pool(name="atn_keep", bufs=1) as keepp:
        # per-head DoubleRow score layout: partition 32*(h%4)+j holds, at
        # [hq=h//4][dh], feature dim 32*dh+j of head h.
        kT8 = keepp.tile([P, 3, 2, T], F8, tag="kT8")
        qT8 = keepp.tile([P, 3, 2, TQ], F8, tag="qT8")
        # V, flat overlapped layout: [key%64, kh, (kb,h)*66 + pad]: each
        # (kb,h) block is [64 v-dims | ones | zero]; the AV stationary reads
        # a 128-wide window so columns 66.. alias the next block — those
        # products land in psum rows 66..127, which are never read.
        VL = NTB * NH * 66
        vp8 = keepp.tile([64, 2, VL + 64], F8, tag="vp8")
        # attn out in proj DoubleRow pairs: chunk 2c+i = head pair p
        aT8 = keepp.tile([P, KP, 2, TQ], F8, tag="aT8")
        bq8 = keepp.tile([P, 3, 2, 1], F32, tag="bq8")

        vp8b = vp8[:, :, 0:VL].rearrange("p k (b c) -> p k b c", c=66)
        nc.gpsimd.memset(vp8b[:, :, :, 64:65], 1.0)
        nc.gpsimd.memset(vp8b[:, :, :, 65:66], 0.0)
        nc.gpsimd.memset(vp8[:, :, VL:], 0.0)

        mlp_w = {}

        def load_mlp_weights(which):
            # on the sync queue, emitted mid-V-loop: they queue behind the
            # wait-bearing vp8 folds so the x/qkv loads keep the DMA engines
            if which == "fc1":
                wfc1 = keepp.tile([P, KC, HID], BF16, tag="wfc1")
                bfc1 = keepp.tile([P, HC, 1], F32, tag="bfc1")
                nc.sync.dma_start(wfc1[:],
                                  d["wfc1"].rearrange("k p f -> p k f"))
                nc.sync.dma_start(bfc1[:, :, :],
                                  d["bfc1"].rearrange("k p o -> p k o"))
                mlp_w["wfc1"] = wfc1
                mlp_w["bfc1"] = bfc1
            else:
                wfc2 = keepp.tile([P, HC, DIM], BF16, tag="wfc2")
                nc.sync.dma_start(wfc2[:],
                                  d["wfc2"].rearrange("k p f -> p k f"))
                mlp_w["wfc2"] = wfc2
                bfc1r = keepp.tile([1, HC, P], BF16, tag="bfc1r")
                nc.sync.dma_start(bfc1r[:], d["bfc1r"])
                mlp_w["bfc1r"] = bfc1r

        # ---- phase A: LN1 + QKV projections (all DoubleRow fp8) ----
        with tc.tile_pool(name="qkv", bufs=1) as qkvp, \
             tc.tile_pool(name="xtok", bufs=2) as xtokp, \
             tc.tile_pool(name="xhat", bufs=2) as xhp, \
             tc.tile_pool(name="v8t", bufs=3) as v8p, \
             tc.tile_pool(name="ps_qkv", bufs=2, space="PSUM") as ps_qkv:

            wq8s = qkvp.tile([P, 3, 2, KP, 2, P], F8, tag="wq8s")
            wk8s = qkvp.tile([P, 3, 2, KP, 2, P], F8, tag="wk8s")
            wv8s = qkvp.tile([P, KP, 2, DIM], F8, tag="wv8s")
            xT8 = qkvp.tile([P, KP, 2, T], F8, tag="xT8")

            xgs = [None] * (NTB // 2)

            def load_xg(g):
                xg = xtokp.tile([P, 2, DIM], BF16, tag="xt", name=f"xg{g}",
                                bufs=8)
                nc.sync.dma_start(
                    xg[:], d["x_tok"][2 * g:2 * g + 2]
                    .rearrange("t p f -> p t f"))
                xgs[g] = xg

            load_xg(0)
            load_xg(1)
            nc.sync.dma_start(
                wk8s[:], d["wk8"].rearrange(
                    "hq dh kcp pr p f -> p hq dh kcp pr f"))
            nc.sync.dma_start(
                wq8s[:], d["wq8"].rearrange(
                    "hq dh kcp pr p f -> p hq dh kcp pr f"))
            nc.scalar.dma_start(bq8[:], d["bq8"].rearrange(
                "hq dh p o -> p hq dh o"))

            # LN1 without a normalize pass: the transpose is a matmul
            # against D = diag(rs) (applies the scale), and -mu*rs rides in
            # as a K=1 ones-row matmul closing each chunk's accumulation.
            nmall = qkvp.tile([P, NTB], BF16, tag="nmall")

            def ln1_stats(tb, ps_tr):
                st = statp.tile([P, 2, 6], F32, tag="st")
                src_ap = xgs[tb // 2][:, tb % 2, :]
                nc.vector.bn_stats(st[:, 0, :], src_ap[:, 0:384])
                nc.vector.bn_stats(st[:, 1, :], src_ap[:, 384:768])
                ag = statp.tile([P, 2], F32, tag="ag")
                nc.vector.bn_aggr(ag[:], st[:])
                sd = statp.tile([P, 1], F32, tag="sd")
                nc.scalar.activation(sd[:], ag[:, 1:2], AF.Sqrt,
                                     bias=epst[:, :])
                rs = statp.tile([P, 1], F32, tag="rs")
                nc.vector.reciprocal(rs[:], sd[:])
                nc.vector.scalar_tensor_tensor(
                    nmall[:, tb:tb + 1], ag[:, 0:1], -1.0, rs[:],
                    ALU.mult, ALU.mult)
                Dt = xhp.tile([P, P], BF16, tag="Dt", name=f"Dt{tb}",
                              bufs=5)
                nc.gpsimd.affine_select(
                    Dt[:], rs[:, 0:1].broadcast_to([128, 128]),
                    pattern=[[-1, 128]], compare_op=ALU.is_equal, fill=0.0,
                    base=0, channel_multiplier=1)
                return Dt

            def ln_transpose(tb, Dt, nmrow, ps_tr):
                ptb = ps_tr.tile([P, KP, 2, P], F32, tag="pt",
                                 name=f"pt{tb}")
                for kc in range(KC):
                    nc.tensor.matmul(
                        ptb[:, kc // 2, kc % 2, :],
                        xgs[tb // 2][:, tb % 2, kc * 128:(kc + 1) * 128],
                        Dt[:], start=True, stop=False)
                    nc.tensor.matmul(
                        ptb[:, kc // 2, kc % 2, :], ones1[0:1, :],
                        nmrow[0:1, :, 0], start=False, stop=True)
                nc.scalar.activation(
                    xT8[:, :, :, tb * 128:(tb + 1) * 128], ptb[:],
                    AF.Identity)

            def kq_proj(dst, wsrc, tsl, with_bias):
                n = tsl.stop - tsl.start
                for hq in range(3):
                    for dh in range(2):
                        ps = ps_qkv.tile([P, 512], F32, tag="qk",
                                         name=f"kq{hq}_{dh}_{tsl.start}_{with_bias}")
                        for kcp in range(KP):
                            nc.tensor.matmul(
                                ps[:, 0:n], wsrc[:, hq, dh, kcp, :, :],
                                xT8[:, kcp, :, tsl],
                                start=(kcp == 0), stop=(kcp == KP - 1),
                                perf_mode=DR,
                            )
                        if with_bias:
                            nc.scalar.activation(
                                dst[:, hq, dh, tsl], ps[:, 0:n],
                                AF.Identity, bias=bq8[:, hq, dh, :])
                        else:
                            nc.vector.tensor_copy(dst[:, hq, dh, tsl],
                                                  ps[:, 0:n])

            def v_proj(tb, ps_v):
                psv = ps_v.tile([P, DIM], F32, tag="psv", name=f"psv{tb}",
                                bufs=2)
                for kcp in range(KP):
                    nc.tensor.matmul(
                        psv[:, 0:512],
                        xT8[:, kcp, :, tb * 128:(tb + 1) * 128],
                        wv8s[:, kcp, :, 0:512],
                        start=(kcp == 0), stop=(kcp == KP - 1),
                        perf_mode=DR,
                    )
                    nc.tensor.matmul(
                        psv[:, 512:768],
                        xT8[:, kcp, :, tb * 128:(tb + 1) * 128],
                        wv8s[:, kcp, :, 512:768],
                        start=(kcp == 0), stop=(kcp == KP - 1),
                        perf_mode=DR,
                    )
                v8 = v8p.tile([P, NH, HD], F8, tag="v8", name=f"v8_{tb}")
                nc.vector.tensor_copy(v8[:], psv[:].rearrange(
                    "p (h e) -> p h e", e=HD))
                vtb = vp8[:, :, tb * NH * 66:(tb + 1) * NH * 66].rearrange(
                    "p k (h c) -> p k h c", c=66)
                nc.sync.dma_start(vtb[:, 0, :, 0:64], v8[0:64, :, :])
                nc.sync.dma_start(vtb[:, 1, :, 0:64], v8[64:128, :, :])

            with tc.tile_pool(name="ps_tr", bufs=1,
                              space="PSUM") as ps_tr, \
                 tc.tile_pool(name="ps_v", bufs=2, space="PSUM") as ps_v:
                for c4 in range(4):
                    if c4 < 3:
                        load_xg(2 * c4 + 2)
                        load_xg(2 * c4 + 3)
                    for tb in range(4 * c4, 4 * c4 + 4):
                        Dt = ln1_stats(tb, ps_tr)
                        nmrow = xhp.tile([1, P, 1], BF16, tag="nmrow",
                                         name=f"nmrow{tb}", bufs=3)
                        nc.sync.dma_start(nmrow[:], nmall[:, tb:tb + 1])
                        ln_transpose(tb, Dt, nmrow, ps_tr)
                    tsl = slice(c4 * 512, c4 * 512 + 512)
                    kq_proj(kT8, wk8s, tsl, False)
                    if c4 < 2:
                        kq_proj(qT8, wq8s, tsl, True)
                    if c4 == 0:
                        nc.scalar.dma_start(
                            wv8s[:],
                            d["wv8"].rearrange("kcp pr p f -> p kcp pr f"))
                for tb in range(NTB):
                    v_proj(tb, ps_v)
                    if tb == 2:
                        load_mlp_weights("fc1")
                    if tb == 6:
                        load_mlp_weights("fc2")
                # preload the exp table so the swap is off the window start
                nc.scalar.activation(sqwarm[:, :], epst[0:1, :], AF.Exp)

            dump("d_xT8", xT8[:])
            dump("d_kT8", kT8[:])
            dump("d_qT8", qT8[:])
            dump("d_vp8", vp8[:])

        # ---- phases B-E: attention groups with interleaved MLP ----
        with tc.tile_pool(name="exs", bufs=2) as exsp, \
             tc.tile_pool(name="exu", bufs=1) as exup, \
             tc.tile_pool(name="dnp", bufs=2) as dnp, \
             tc.tile_pool(name="recb", bufs=2) as recbp, \
             tc.tile_pool(name="xhat2", bufs=2) as xh2p, \
             tc.tile_pool(name="x2tp", bufs=1) as x2tp, \
             tc.tile_pool(name="htp", bufs=1) as htp, \
             tc.tile_pool(name="prp", bufs=1) as prp, \
             tc.tile_pool(name="ps_sc", bufs=2, space="PSUM") as ps_sc, \
             tc.tile_pool(name="ps_av", bufs=2, space="PSUM") as ps_av, \
             tc.tile_pool(name="ps_mi", bufs=2, space="PSUM") as ps_mi:

            wproj8 = prp.tile([P, KP, 2, DIM], F8, tag="wproj8")
            nc.scalar.dma_start(
                wproj8[:], d["wproj8"].rearrange("kcp pr p f -> p kcp pr f"))

            def attn_group(p, qc):
                hA, hB = 2 * p, 2 * p + 1
                qsl = slice(qc * 512, (qc + 1) * 512)
                avA = ps_av.tile([128, 512], F32, tag="av",
                                 name=f"avA{p}_{qc}")
                avB = ps_av.tile([128, 512], F32, tag="av",
                                 name=f"avB{p}_{qc}")
                for pack in range(4):
                    exS = exsp.tile([P, 4, 2, 512], F8, tag="exS",
                                    name=f"exS{p}_{qc}_{pack}")
                    exU = exup.tile([64, 4, 2, 2, 512], F8, tag="exU",
                                    name=f"exU{p}_{qc}_{pack}")
                    for kbi in range(4):
                        kb = 4 * pack + kbi
                        ksl = slice(kb * 128, (kb + 1) * 128)
                        psS = ps_sc.tile([P, 2, 512], F32, tag="sc",
                                         name=f"sc{p}_{qc}_{kb}")
                        for hh, h in ((0, hA), (1, hB)):
                            s = h % 4
                            hq = h // 4
                            nc.tensor.matmul(
                                psS[:, hh, :],
                                kT8[32 * s:32 * s + 32, hq, :, ksl],
                                qT8[32 * s:32 * s + 32, hq, :, qsl],
                                perf_mode=DR, tile_position=(32 * s, 0),
                            )
                        nc.scalar.activation(exS[:, kbi, :, :], psS[:, :, :],
                                             AF.Exp)
                    nc.sync.dma_start(exU[:, :, :, 0, :], exS[0:64])
                    nc.sync.dma_start(exU[:, :, :, 1, :], exS[64:128])
                    for kbi in range(4):
                        kb = 4 * pack + kbi
                        sA = (kb * NH + hA) * 66
                        sB = (kb * NH + hB) * 66
                        nc.tensor.matmul(
                            avA[:], vp8[:, :, sA:sA + 128],
                            exU[:, kbi, 0, :, :],
                            start=(kb == 0), stop=(kb == NTB - 1),
                            perf_mode=DR,
                        )
                        nc.tensor.matmul(
                            avB[:], vp8[:, :, sB:sB + 128],
                            exU[:, kbi, 1, :, :],
                            start=(kb == 0), stop=(kb == NTB - 1),
                            perf_mode=DR,
                        )
                return avA, avB

            def normalize(p, qc, avA, avB):
                qsl = slice(qc * 512, (qc + 1) * 512)
                dnrow = dnp.tile([1, 2, 512], BF16, tag="dnrow",
                                 name=f"dnrow{p}_{qc}")
                with nc.allow_low_precision(reason="softmax denom recip"):
                    nc.vector.reciprocal(dnrow[0:1, 0, :], avA[64:65, :])
                    nc.vector.reciprocal(dnrow[0:1, 1, :], avB[64:65, :])
                recb = recbp.tile([64, 2, 512], BF16, tag="recb",
                                  name=f"recb{p}_{qc}")
                nc.gpsimd.partition_broadcast(recb[:], dnrow[0:1, :, :])
                c, i = p // 2, p % 2
                nc.vector.tensor_tensor(
                    aT8[0:64, c, i, qsl], avA[0:64, :], recb[:, 0, :],
                    op=ALU.mult)
                nc.vector.tensor_tensor(
                    aT8[64:128, c, i, qsl], avB[0:64, :], recb[:, 1, :],
                    op=ALU.mult)

            def proj_ln2_x2t(qc, x2T, pool, tag):
                # phase-split across the 4 mbs so the ACT sqrts batch and
                # the per-mb chains pipeline across engines
                for mbq in range(4):
                    mb = 4 * qc + mbq
                    msl = slice(mb * 128, (mb + 1) * 128)
                    xres = recbp.tile([P, DIM], BF16, tag="xres",
                                      name=f"xres{mb}", bufs=1)
                    nc.scalar.dma_start(xres[:], d["x_res"][mb])
                    ps0 = pool.tile([P, 512], F32, tag=tag,
                                    name=f"pj0_{mb}")
                    ps1 = pool.tile([P, 256], F32, tag=tag,
                                    name=f"pj1_{mb}")
                    for kcp in range(KP):
                        nc.tensor.matmul(
                            ps0[:], aT8[:, kcp, :, msl],
                            wproj8[:, kcp, :, 0:512],
                            start=(kcp == 0), stop=(kcp == KP - 1),
                            perf_mode=DR,
                        )
                        nc.tensor.matmul(
                            ps1[:], aT8[:, kcp, :, msl],
                            wproj8[:, kcp, :, 512:768],
                            start=(kcp == 0), stop=(kcp == KP - 1),
                            perf_mode=DR,
                        )
                    nc.vector.tensor_tensor(y1[:, mb, 0:512], ps0[:],
                                            xres[:, 0:512], op=ALU.add)
                    nc.vector.tensor_tensor(y1[:, mb, 512:768], ps1[:],
                                            xres[:, 512:768], op=ALU.add)
                for half in range(2):
                    stats = [ln_stats(y1[:, 4 * qc + 2 * half + j, :], False)
                             for j in range(2)]
                    for j in range(2):
                        mbq = 2 * half + j
                        mb = 4 * qc + mbq
                        rs, nm = stats[j]
                        xh2 = xh2p.tile([P, DIM], BF16, tag="xh2",
                                        name=f"xh2_{mb}", bufs=1)
                        nc.vector.tensor_scalar(
                            xh2[:], y1[:, mb, :], rs[:], nm[:],
                            ALU.mult, ALU.add)
                        pt2 = pool.tile([P, KC, P], BF16, tag=tag,
                                        name=f"p2t{mb}")
                        for kc in range(KC):
                            nc.tensor.transpose(
                                pt2[:, kc, :],
                                xh2[:, kc * 128:(kc + 1) * 128],
                                ident16[:, :])
                        nc.vector.tensor_copy(
                            x2T[:, :, mbq * 128:(mbq + 1) * 128], pt2[:])

            def fc1_block(hb, x2T, hT, pool, tag):
                """fc1 matmul chain; raw (biased, un-gelu'd) evict to hT."""
                wfc1, bfc1 = mlp_w["wfc1"], mlp_w["bfc1"]
                ps = pool.tile([P, 512], F32, tag=tag, name=f"fc1_{hb}")
                for kc in range(KC):
                    nc.tensor.matmul(
                        ps[:], wfc1[:, kc, hb * 128:(hb + 1) * 128],
                        x2T[:, kc, :],
                        start=(kc == 0), stop=(kc == KC - 1),
                    )
                nc.vector.tensor_scalar(hT[:, hb, :], ps[:],
                                        bfc1[:, hb, :], None, ALU.add)

            def gelu_batch(hT, lo, hi):
                nc.scalar.activation(hT[:, lo:hi, :], hT[:, lo:hi, :],
                                     AF.Gelu)

            def fc2_block(qc, mbq, hT, pool, tag):
                wfc2 = mlp_w["wfc2"]
                mb = 4 * qc + mbq
                msl = slice(mbq * 128, (mbq + 1) * 128)
                ps0 = pool.tile([P, 512], F32, tag=tag, name=f"f20_{mb}")
                ps1 = pool.tile([P, 256], F32, tag=tag, name=f"f21_{mb}")
                for kc in range(HC):
                    nc.tensor.matmul(ps0[:], hT[:, kc, msl],
                                     wfc2[:, kc, 0:512],
                                     start=(kc == 0), stop=False)
                    nc.tensor.matmul(ps1[:], hT[:, kc, msl],
                                     wfc2[:, kc, 512:768],
                                     start=(kc == 0), stop=False)
                nc.tensor.matmul(ps0[:], ones1[0:1, :],
                                 bfc2[0:1, 0:512], start=False, stop=True)
                nc.tensor.matmul(ps1[:], ones1[0:1, :],
                                 bfc2[0:1, 512:768], start=False, stop=True)
                yo = yop.tile([P, DIM], F32, tag="yo")
                nc.vector.tensor_tensor(yo[:, 0:512], ps0[:],
                                        y1[:, mb, 0:512], op=ALU.add)
                nc.vector.tensor_tensor(yo[:, 512:768], ps1[:],
                                        y1[:, mb, 512:768], op=ALU.add)
                nc.sync.dma_start(d["y_out"][mb], yo[:])

            # -------- qc0 attention (PE hole: weight loads only) --------
            for p in range(HP):
                avA, avB = attn_group(p, 0)
                normalize(p, 0, avA, avB)

            x2T0 = x2tp.tile([P, KC, 512], BF16, tag="x2T", name="x2T0")
            hT0 = htp.tile([P, HC, 512], BF16, tag="hT", name="hT0")

            # -------- qc1 attention with proj0 + MLP(qc0) interleaved ----
            for p in range(HP):
                avA, avB = attn_group(p, 1)
                normalize(p, 1, avA, avB)
                if p == 0:
                    proj_ln2_x2t(0, x2T0, ps_mi, "mi")
                elif p == 1:
                    for hb in range(0, 16):
                        fc1_block(hb, x2T0, hT0, ps_mi, "mi")
                elif p == 2:
                    for hb in range(16, HC):
                        fc1_block(hb, x2T0, hT0, ps_mi, "mi")
                    gelu_batch(hT0, 0, 12)
                    gelu_batch(hT0, 12, HC)
                    fc2_block(0, 0, hT0, ps_mi, "mi")
                else:
                    fc2_block(0, p - 2, hT0, ps_mi, "mi")

            # -------- tail: proj/LN2/x2T + MLP for qc1 --------
            # attention psum pools are drained now; borrow ps_sc for depth
            x2T1 = x2tp.tile([P, KC, 512], BF16, tag="x2T", name="x2T1")
            hT1 = htp.tile([P, HC, 512], BF16, tag="hT", name="hT1")
            proj_ln2_x2t(1, x2T1, ps_sc, "sc")
            for hb in range(HC):
                if hb % 2:
                    fc1_block(hb, x2T1, hT1, ps_sc, "sc")
                else:
                    fc1_block(hb, x2T1, hT1, ps_mi, "mi")
            gelu_batch(hT1, 0, 12)
            gelu_batch(hT1, 12, HC)
            for mbq in range(4):
                if mbq % 2:
                    fc2_block(1, mbq, hT1, ps_sc, "sc")
                else:
                    fc2_block(1, mbq, hT1, ps_mi, "mi")

            dump("d_aT8", aT8[:])
            if DEBUG_DUMPS:
                for mb in range(NQB):
                    nc.sync.dma_start(d["d_y1"][mb], y1[:, mb, :])


_PROGRAM = None


def build_program():
    global _PROGRAM
    if _PROGRAM is not None:
        return _PROGRAM
    nc = bacc.Bacc("TRN2", debug=False, target_bir_lowering=False,
                   num_devices=NCORES)
    d = {}

    def din(name, shape, dt):
        d[name] = nc.dram_tensor(name, shape, dt, kind="ExternalInput").ap()

    din("x_tok", [NTB, 128, DIM], BF16)
    din("x_res", [NQB, 128, DIM], BF16)
    din("wq8", [3, 2, KP, 2, 128, 128], F8)
    din("wk8", [3, 2, KP, 2, 128, 128], F8)
    din("bq8", [3, 2, 128, 1], F32)
    din("wv8", [KP, 2, 128, DIM], F8)
    din("wproj8", [KP, 2, 128, DIM], F8)
    din("wfc1", [KC, 128, HID], BF16)
    din("bfc1", [HC, 128, 1], F32)
    din("bfc1r", [1, HC, 128], BF16)
    din("wfc2", [HC, 128, DIM], BF16)
    din("bfc2", [1, DIM], BF16)
    din("ident16", [128, 128], BF16)
    d["y_out"] = nc.dram_tensor("y_out", [NQB, 128, DIM], F32,
                                kind="ExternalOutput").ap()
    if DEBUG_DUMPS:
        def dout(name, shape, dt):
            d[name] = nc.dram_tensor(name, shape, dt,
                                     kind="ExternalOutput").ap()
        dout("d_xT8", [128, KP, 2, T], F8)
        dout("d_kT8", [128, 3, 2, T], F8)
        dout("d_qT8", [128, 3, 2, TQ], F8)
        dout("d_vp8", [64, 2, NTB * NH * 66 + 64], F8)
        dout("d_aT8", [128, KP, 2, TQ], F8)
        dout("d_y1", [NQB, 128, DIM], F32)

    with tile.TileContext(nc) as tc:
        with ExitStack() as ctx:
            _emit(nc, tc, ctx, d)
    nc.compile()
    _PROGRAM = nc
    return nc


def _score_layout_weight(W):
    """W [768 out, 768 in] -> [hq 3, dh 2, kcp 3, pr 2, p 128, f 128] where
    out feature 64*(4hq+s)+32dh+j -> f = 32s+j, in dim 128*(2kcp+pr)+p."""
    A = W.T  # [in 768, out 768]
    arr = A.reshape(KP, 2, 128, 3, 4, 2, 32)   # kcp pr p hq s dh j
    arr = arr.transpose(3, 5, 0, 1, 2, 4, 6)   # hq dh kcp pr p s j
    return np.ascontiguousarray(arr.reshape(3, 2, KP, 2, 128, 128))


def _prep_in_maps(inputs):
    f32 = lambda a: np.ascontiguousarray(np.asarray(a, dtype=np.float32))
    bf = lambda a: np.ascontiguousarray(
        np.asarray(a, dtype=np.float32).astype(ml_dtypes.bfloat16))
    f8 = lambda a: np.ascontiguousarray(
        np.asarray(a, dtype=np.float32).astype(E4))

    x = f32(inputs["x"])
    g1, b1 = f32(inputs["ln1_g"]), f32(inputs["ln1_b"])
    qkv_w, qkv_b = f32(inputs["qkv_w"]), f32(inputs["qkv_b"])
    proj_w, proj_b = f32(inputs["proj_w"]), f32(inputs["proj_b"])
    g2, b2 = f32(inputs["ln2_g"]), f32(inputs["ln2_b"])
    fc1_w, fc1_b = f32(inputs["fc1_w"]), f32(inputs["fc1_b"])
    fc2_w, fc2_b = f32(inputs["fc2_w"]), f32(inputs["fc2_b"])

    Wq, Wk, Wv = qkv_w[:DIM], qkv_w[DIM:2 * DIM], qkv_w[2 * DIM:]
    scale = float(HD) ** -0.5
    bq_eff = (qkv_b[:DIM] + Wq @ b1) * scale
    bv_eff = qkv_b[2 * DIM:] + Wv @ b1
    xres_const = proj_b + proj_w @ bv_eff

    bq_l = bq_eff.reshape(3, 4, 2, 32).transpose(0, 2, 1, 3)  # hq dh s j
    # wproj rows in aT8 layout: chunk 2c+i = head pair, parts 0:64 head 2p,
    # 64:128 head 2p+1; feature-major within head.
    wproj_l = proj_w.T.reshape(KP, 2, 128, DIM)

    shared = {
        "ident16": bf(np.eye(128, dtype=np.float32)),
        "wq8": f8(_score_layout_weight(Wq * g1 * scale)),
        "wk8": f8(_score_layout_weight(Wk * g1)),
        "bq8": f32(np.ascontiguousarray(bq_l.reshape(3, 2, 128, 1))),
        "wv8": f8((Wv * g1).T.reshape(KP, 2, 128, DIM)),
        "wproj8": f8(np.ascontiguousarray(wproj_l)),
        "wfc1": bf((fc1_w * g2).T.reshape(KC, 128, HID)),
        "bfc1": f32((fc1_b + fc1_w @ b2).reshape(HC, 128, 1)),
        "bfc1r": bf((fc1_b + fc1_w @ b2).reshape(1, HC, 128)),
        "wfc2": bf(fc2_w.T.reshape(HC, 128, DIM)),
        "bfc2": bf(fc2_b.reshape(1, DIM)),
    }
    in_maps = []
    for c in range(NCORES):
        b, h = divmod(c, 2)
        xr = np.roll(x[b], -h * TQ, axis=0)
        m = dict(shared)
        m["x_tok"] = bf(xr.reshape(NTB, 128, DIM))
        m["x_res"] = bf((xr[:TQ] + xres_const).reshape(NQB, 128, DIM))
        in_maps.append(m)
    return in_maps


def run(inputs, trace=False, **kwargs):
    nc = build_program()
    in_maps = _prep_in_maps(inputs)
    res = run_bass_kernel_spmd(nc, in_maps, core_ids=list(range(NCORES)),
                               trace=trace, **kwargs)
    out = np.empty((B, T, DIM), np.float32)
    for c in range(NCORES):
        b, h = divmod(c, 2)
        out[b, h * TQ:(h + 1) * TQ] = (
            res.results[c]["y_out"].reshape(TQ, DIM).astype(np.float32))
    return out, res


def kernel(**inputs) -> np.ndarray:
    out, _ = run(inputs, trace=False)
    return out
